# revision 36
# baseline (speedup 1.0000x reference)
"""CQAttention Trainium2 kernel.

Math (per batch b):
  S = (C*w3) @ Q^T + (C@w1)[:,None] + (Q@w2)[None,:] (+bias, dropped: softmax-invariant)
  Sq = softmax over q of qmask-masked S ; Sc = softmax over c of cmask-masked S
  A = Sq@Q ; Bm = Sq @ (Sc^T @ C) ; out = [C | A | C*A | C*Bm]

Device algorithm. No max-subtraction (|S| < 1 so exp is safe). All rank-1
bias/mask factors are precomputed on host as multiplicative exp factors and
folded into matmul operands, so the device only computes:
  ST  = (Q*w3) @ C^T                [q, c]   (PE, fp32r)
  X   = exp(ST)                     [q, c]
  XTg = X^T * g[c]                  [c, q]   g = exp(C@w1 + cneg), fold in the
                                             PSUM->SBUF copy of the transpose
  T1  = XTg^T @ [C|1]               [q, d+2] numerator | W (col denominators)
  T1g = T1[:, :d] * (eb/W)[q]       [q, d]   eb = exp(Q@w2 + qneg)
  psA = X^T @ [Q*eb | eb]           [c, d+2] unnormalized A | Z (row denoms)
  psB = X^T @ T1g                   [c, d]   unnormalized Bm
  A = psA/Z ; CA = C*A ; CBm = C*psB/Z

Sharding: data-parallel over batch, 4 batches per core on 8 cores.
"""

import numpy as np

NEG_INF = -1e30
B_FULL, LC, LQ, D = 32, 1024, 128, 256
D2 = D + 2
N_CORES = 8
NB = B_FULL // N_CORES  # batches per core
KC = LC // 128  # c-tiles per batch (8)

_CACHE = {}


def _build_nc():
    import concourse.bacc as bacc
    import concourse.mybir as mybir
    from concourse import tile
    from concourse.masks import make_identity

    fp32 = mybir.dt.float32
    fp32r = mybir.dt.float32r
    MULT = mybir.AluOpType.mult
    EXP = mybir.ActivationFunctionType.Exp

    nc = bacc.Bacc("TRN2", target_bir_lowering=False, debug=False)

    # Cp is host-relaid to partition-major [NB, 128(p), KC, D2] so each
    # SBUF partition line is one contiguous 8KB DRAM read.
    C_d = nc.dram_tensor("Cp", [NB, 128, KC, D2], fp32, kind="ExternalInput")
    # aux packs qt3 | qg | meta along the free axis (one load):
    # [128, NB, 2*128 + D2 + KC+1]
    AUXW = 2 * 128 + D2 + (KC + 1)
    MOFF = 2 * 128 + D2
    aux_d = nc.dram_tensor("aux", [128, NB, AUXW], fp32, kind="ExternalInput")
    out_d = nc.dram_tensor("out", [NB, LC, 4 * D], fp32, kind="ExternalOutput")

    with tile.TileContext(nc) as tc:
        with (
            tc.tile_pool(name="const", bufs=1) as const,
            tc.tile_pool(name="cpool", bufs=NB) as p_c,
            tc.tile_pool(name="crpool", bufs=2) as p_cr,
            tc.tile_pool(name="ctpool", bufs=2) as p_ct,
            tc.tile_pool(name="xpool", bufs=2) as p_x,
            tc.tile_pool(name="xtpool", bufs=2) as p_xt,
            tc.tile_pool(name="t1gpool", bufs=2) as p_t1g,
            tc.tile_pool(name="smpool", bufs=6) as p_sm,
            tc.tile_pool(name="opool", bufs=8) as p_o,
            tc.tile_pool(name="pstr", bufs=3, space="PSUM") as ps_tr,
            tc.tile_pool(name="pst1", bufs=1, space="PSUM") as ps_t1,
            tc.tile_pool(name="psa", bufs=2, space="PSUM") as ps_a,
            tc.tile_pool(name="psb", bufs=2, space="PSUM") as ps_b,
        ):
            ident = const.tile([128, 128], fp32)
            make_identity(nc, ident)
            identr = const.tile([128, 128], fp32r)
            nc.vector.tensor_copy(identr, ident)

            # ---- hoisted input loads (first C tile first, so PE can start) ----
            # Loads go out on the ACT HWDGE ring (nc.scalar) so the store
            # stream on the SP ring (nc.sync) can overlap them.
            C1s = [None] * NB
            C1s[0] = p_c.tile([128, KC, D2], fp32, tag="c", name="c1_first")
            nc.scalar.dma_start(C1s[0], C_d.ap()[0])

            auxf = const.tile([128, NB, AUXW], fp32)
            nc.scalar.dma_start(auxf, aux_d.ap())

            for b in range(1, NB):
                C1s[b] = p_c.tile([128, KC, D2], fp32, tag="c", name="c1")
                nc.scalar.dma_start(C1s[b], C_d.ap()[b])

            # C passthrough stores on the SP ring: overlap the tail of the
            # loads and bridge the FIFO until compute stores are ready.
            for b in range(NB):
                nc.sync.dma_start(
                    out_d.ap()[b, :, 0:D].rearrange("(k p) d -> p k d", p=128),
                    C1s[b][:, :, 0:D],
                )

            # one-time fp32r rounding of the shared matmul operands
            qt3 = const.tile([128, NB, 256], fp32r)
            nc.vector.tensor_copy(qt3, auxf[:, :, 0:256])
            qg = const.tile([128, NB, D2], fp32r)
            nc.scalar.copy(qg, auxf[:, :, 256 : 256 + D2])



            for b in range(NB):
                C1 = C1s[b]

                # ---- rounded [C|1] copy for the T1 rhs ----
                Cr = p_cr.tile([128, KC, D2], fp32r, tag="cr")
                nc.scalar.copy(Cr[:, 0 : KC // 2], C1[:, 0 : KC // 2])
                nc.vector.tensor_copy(Cr[:, KC // 2 : KC], C1[:, KC // 2 : KC])

                # ---- CT = C^T per d-chunk ----
                CT = p_ct.tile([128, 2, LC], fp32r, tag="ct")
                for dk in range(2):
                    for h in range(2):
                        pt = ps_tr.tile([128, 512], fp32, tag="pt")
                        for j in range(4):
                            k = h * 4 + j
                            nc.tensor.transpose(
                                pt[:, j * 128 : (j + 1) * 128],
                                C1[:, k, dk * 128 : (dk + 1) * 128],
                                ident,
                            )
                        dst = CT[:, dk, h * 512 : (h + 1) * 512]
                        if (dk * 2 + h) % 2 == 0:
                            nc.scalar.copy(dst, pt)
                        else:
                            nc.vector.tensor_copy(dst, pt)

                # ---- ST = (Q*w3) @ C^T ; X = exp(ST) ----
                X = p_x.tile([128, LC], fp32r, tag="x")
                for h in range(2):
                    st = ps_tr.tile([128, 512], fp32, tag="pt")
                    for dk in range(2):
                        nc.tensor.matmul(
                            st,
                            qt3[:, b, dk * 128 : (dk + 1) * 128],
                            CT[:, dk, h * 512 : (h + 1) * 512],
                            start=(dk == 0),
                            stop=(dk == 1),
                        )
                    nc.scalar.activation(X[:, h * 512 : (h + 1) * 512], st, EXP)

                # ---- XTg = X^T * g (g folded into the PSUM->SBUF copy) ----
                XT = p_xt.tile([128, KC, 128], fp32r, tag="xt")
                for h in range(2):
                    pt = ps_tr.tile([128, 512], fp32r, tag="pt")
                    for j in range(4):
                        k = h * 4 + j
                        nc.tensor.transpose(
                            pt[:, j * 128 : (j + 1) * 128],
                            X[:, k * 128 : (k + 1) * 128],
                            identr,
                        )
                    for j in range(4):
                        k = h * 4 + j
                        src = pt[:, j * 128 : (j + 1) * 128]
                        gk = auxf[:, b, MOFF + k : MOFF + k + 1]
                        if j % 2 == 0:
                            nc.scalar.mul(XT[:, k], src, gk)
                        else:
                            nc.vector.tensor_scalar_mul(XT[:, k], src, gk)

                # ---- T1 = XTg^T @ [C|1] ; T1g = T1 * (eb/W) ----
                t1 = ps_t1.tile([128, D2], fp32, tag="t1")
                for k in range(KC):
                    nc.tensor.matmul(
                        t1,
                        XT[:, k],
                        Cr[:, k],
                        start=(k == 0),
                        stop=(k == KC - 1),
                    )
                recipT = p_sm.tile([128, 1], fp32, tag="recipT")
                nc.vector.reciprocal(recipT, t1[:, D : D + 1])
                scal = p_sm.tile([128, 1], fp32, tag="scal")
                nc.vector.tensor_mul(scal, recipT, auxf[:, b, MOFF + KC : MOFF + KC + 1])
                T1g = p_t1g.tile([128, D], fp32r, tag="t1g")
                nc.vector.tensor_scalar_mul(T1g, t1[:, 0:D], scal)

                # ---- per c-tile: psA / psB -> A / CA / CBm ----
                psAs = [None] * KC
                psAs[0] = ps_a.tile([128, D2], fp32, tag="psa", name="psa0")
                nc.tensor.matmul(psAs[0], X[:, 0:128], qg[:, b], start=True, stop=True)
                for k in range(KC):
                    kk = k % 2
                    if kk == 0:
                        osb = p_o.tile([128, 2, 3 * D], fp32, tag="osb")
                    if k + 1 < KC:
                        psAs[k + 1] = ps_a.tile([128, D2], fp32, tag="psa", name="psa")
                        nc.tensor.matmul(
                            psAs[k + 1],
                            X[:, (k + 1) * 128 : (k + 2) * 128],
                            qg[:, b],
                            start=True,
                            stop=True,
                        )
                    psA = psAs[k]
                    psB = ps_b.tile([128, D], fp32, tag="psb")
                    nc.tensor.matmul(
                        psB, X[:, k * 128 : (k + 1) * 128], T1g, start=True, stop=True
                    )

                    rr = p_sm.tile([128, 1], fp32, tag="rr")
                    nc.vector.reciprocal(rr, psA[:, D : D + 1])

                    # A = psA * rr  (per-partition scale, alternate ACT/DVE)
                    if kk == 0:
                        nc.scalar.mul(osb[:, kk, 0:D], psA[:, 0:D], rr)
                    else:
                        nc.vector.tensor_scalar_mul(osb[:, kk, 0:D], psA[:, 0:D], rr)
                    # CA = C * A  (GPSIMD; reads the extracted A from SBUF)
                    nc.gpsimd.tensor_mul(
                        osb[:, kk, D : 2 * D], C1[:, k, 0:D], osb[:, kk, 0:D]
                    )
                    # CBm = (psB * rr) * C  (DVE fused)
                    nc.vector.scalar_tensor_tensor(
                        osb[:, kk, 2 * D : 3 * D], psB, rr, C1[:, k, 0:D], MULT, MULT
                    )
                    if kk == 1:
                        nc.sync.dma_start(
                            out_d.ap()[
                                b, (k - 1) * 128 : (k + 1) * 128, D : 4 * D
                            ].rearrange("(k p) n -> p k n", p=128),
                            osb,
                        )


    nc.compile()
    return nc


def _get_nc():
    if "nc" not in _CACHE:
        _CACHE["nc"] = _build_nc()
    return _CACHE["nc"]


def _make_in_maps(C, Q, cmask, qmask, Wo_w):
    C = np.ascontiguousarray(C, dtype=np.float32)
    Q = np.ascontiguousarray(Q, dtype=np.float32)
    w = np.asarray(Wo_w, dtype=np.float32)
    w1, w2, w3 = w[:D], w[D : 2 * D], w[2 * D :]

    rc = (C @ w1).astype(np.float32)  # [B, Lc]
    rq = (Q @ w2).astype(np.float32)  # [B, Lq]
    cneg = ((1.0 - cmask.astype(np.float32)) * NEG_INF).astype(np.float32)
    qneg = ((1.0 - qmask.astype(np.float32)) * NEG_INF).astype(np.float32)
    with np.errstate(under="ignore", over="ignore"):
        g = np.exp(rc + cneg).astype(np.float32)  # [B, Lc]
        eb = np.exp(rq + qneg).astype(np.float32)  # [B, Lq]

    ones2 = np.ones((B_FULL, LC, 2), np.float32)
    Cp = np.concatenate([C, ones2], axis=2)  # [B, Lc, 258]
    # partition-major relayout: [B, KC, 128(p), D2] -> [B, 128(p), KC, D2]
    Cp = Cp.reshape(B_FULL, KC, 128, D2).transpose(0, 2, 1, 3)

    QT3 = (Q.transpose(0, 2, 1) * w3[None, :, None]).reshape(B_FULL, 2, 128, LQ)
    QT3 = QT3.transpose(0, 2, 1, 3).reshape(B_FULL, 128, 256)  # [B, 128(p), dk*q]

    ebc = eb[:, :, None]
    Qg = np.concatenate([Q * ebc, ebc, ebc], axis=2)  # [B, 128, 258]

    gm = g.reshape(B_FULL, KC, 128).transpose(0, 2, 1)  # [B, 128, KC]
    meta = np.concatenate([gm, eb[:, :, None]], axis=2)  # [B, 128, KC+1]

    aux = np.concatenate([QT3, Qg, meta], axis=2)  # [B, 128, 523]

    in_maps = []
    for i in range(N_CORES):
        sl = slice(i * NB, (i + 1) * NB)
        in_maps.append(
            {
                "Cp": np.ascontiguousarray(Cp[sl]),
                "aux": np.ascontiguousarray(aux[sl].transpose(1, 0, 2)),
            }
        )
    return in_maps


def kernel(C, Q, cmask, qmask, Wo_w, Wo_b):
    from concourse.bass_utils import run_bass_kernel_spmd

    nc = _get_nc()
    in_maps = _make_in_maps(C, Q, cmask, qmask, Wo_w)
    res = run_bass_kernel_spmd(nc, in_maps, core_ids=list(range(N_CORES)))
    out = np.concatenate([res.results[i]["out"] for i in range(N_CORES)], axis=0)
    return out
